# revision 19
# baseline (speedup 1.0000x reference)
"""Trainium2 Bass kernel: multi-head attention block (B=4, N=2048, C=1024, H=16).

Sharding: 8 cores = (batch b in 0..3) x (head-group hg in 0..1, 8 heads each).
Each core computes qkv for its heads, full attention for its heads over its
batch, and a partial projection (its 512 rows of W_proj). Host sums the two
partials per batch and adds b_proj.

Device layout (all matmuls bf16 inputs, fp32 PSUM accumulate):
  - q,k produced in transposed layout qkT[dim, token] so S^T = k-chunk.T @ q^T
    needs no on-chip transposes.
  - v produced in natural layout [token, 65*h] with a trailing ones column per
    head, so the O matmul lhsT=[v|ones] gives row 64 = softmax denominator and
    rows 0..63 = unnormalized o^T in one PSUM accumulation chain.
  - head PAIRS: the two heads' S matmuls sit at base partitions 0/64 (distinct
    PE row groups) and issue back-to-back, so they run concurrently.
  - exp alternates between ACT (exact, fused scale 1/8) and DVE (Schraudolph
    bit-trick: bits = round(S*log2e*16 + 16250.5) as int16, bitcast to bf16 =
    2^(S*log2e/8) with ~2% rms error). This keeps the softmax off the critical
    path: neither engine is saturated, so the j-loop period is PE-bound.
    (fp8/DoubleRow was measured to net zero here: DR limits M to 64, which
    costs the ones-row denominator exactly what the doubled contraction saves.)
  - proj quarters are emitted inside pair 3's i4 loop (PE filler), and the
    output store is bf16 (host sums partials in fp32).
"""

import os
import sys
from contextlib import ExitStack

import numpy as np
import ml_dtypes

import concourse.bass as bass
import concourse.tile as tile
from concourse import bacc, mybir
from concourse.bass import ds, ts
from concourse.bass_utils import run_bass_kernel_spmd

try:  # without the NTFF hook module, a stray BASS_TRACE=1 would crash the run
    from antenv.axon_hooks import get_axon_ntff_profile_hook  # noqa: F401
except ImportError:
    os.environ.setdefault("BASS_NEVER_TRACE", "1")

BF16 = mybir.dt.bfloat16
F32 = mybir.dt.float32
I16 = mybir.dt.int16
NP_BF16 = ml_dtypes.bfloat16

B, N, C = 4, 2048, 1024
H, D = 16, 64
HPC = 8            # heads per core
CD = HPC * D       # 512 local qkv dims per core
E = D + 1          # 65: 64 v dims + ones column

LOG2E = 1.4426950408889634
SCH_A = 128.0 * LOG2E / 8.0      # 23.083...
SCH_C = 16256.0 - 5.5            # bf16 exp2 bias with mid-point correction

# per-j exp engine (16 js): A = ACT exact exp, D = DVE schraudolph
ENGINES = "ADADADADADADADAD"

LAST_RESULTS = None  # stash for test harness (exec_time_ns, trace paths)


def _build_program(taps=False):
    nc = bacc.Bacc("TRN2", target_bir_lowering=False, debug=False)

    xT_d = nc.dram_tensor("xT", [C, N], BF16, kind="ExternalInput").ap()
    wqk_d = nc.dram_tensor("wqk", [C, 2 * CD], BF16, kind="ExternalInput").ap()
    wv_d = nc.dram_tensor("wv", [C, CD], BF16, kind="ExternalInput").ap()
    bqk_d = nc.dram_tensor("bqk", [128, 8], F32, kind="ExternalInput").ap()
    bv_d = nc.dram_tensor("bv", [1, CD], BF16, kind="ExternalInput").ap()
    wp_d = nc.dram_tensor("wp", [CD, C], BF16, kind="ExternalInput").ap()
    out_d = nc.dram_tensor("out", [N, C], BF16, kind="ExternalOutput").ap()
    if taps:
        tap_ex = nc.dram_tensor("tap_ex", [128, 1024], BF16, kind="ExternalOutput").ap()
        tap_ot = nc.dram_tensor("tap_ot", [65, 512], F32, kind="ExternalOutput").ap()

    with tile.TileContext(nc) as tc, ExitStack() as ctx:
        singles = ctx.enter_context(tc.tile_pool(name="singles", bufs=1))
        ps_pool = ctx.enter_context(tc.tile_pool(name="ps", bufs=2, space="PSUM"))
        st_pool = ctx.enter_context(tc.tile_pool(name="stp", bufs=2, space="PSUM"))
        ot_pool = ctx.enter_context(tc.tile_pool(name="ot", bufs=2, space="PSUM"))
        exp_pool = ctx.enter_context(tc.tile_pool(name="expp", bufs=4))
        misc = ctx.enter_context(tc.tile_pool(name="misc", bufs=4))
        ob_pool = ctx.enter_context(tc.tile_pool(name="ob", bufs=3))

        # Persistent SBUF tensors, chunk-major: [partition, chunk, free].
        xT_sb = singles.tile([128, 8, N], BF16)        # x^T   [c, token]
        wqk_sb = singles.tile([128, 8, 2 * CD], BF16)  # W_qk  [c, m]
        wv_sb = singles.tile([128, 8, CD], BF16)       # W_v   [c, n]
        bqk_sb = singles.tile([128, 8], F32)
        bv_sb = singles.tile([1, CD], BF16)
        ones_sb = singles.tile([1, 128], BF16)
        wp_sb = singles.tile([128, 4, C], BF16)        # W_proj [hd, n]
        qkT_sb = singles.tile([128, 8, N], BF16)       # chunks 0..3 = q, 4..7 = k
        v_sb = singles.tile([128, 16, HPC * E], BF16)  # [token-in-chunk, tchunk, h*(64+1)]
        oT_sb = singles.tile([128, 4, N], BF16)        # o^T, proj lhsT layout

        nc.sync.dma_start(wv_sb, wv_d.rearrange("(c p) m -> p c m", p=128))
        nc.sync.dma_start(bv_sb, bv_d)
        for kc in range(8):
            nc.sync.dma_start(xT_sb[:, kc], xT_d.rearrange("(c p) t -> p c t", p=128)[:, kc])
        nc.sync.dma_start(wqk_sb, wqk_d.rearrange("(c p) m -> p c m", p=128))
        nc.sync.dma_start(bqk_sb, bqk_d)
        nc.sync.dma_start(wp_sb, wp_d.rearrange("(c p) n -> p c n", p=128))
        nc.vector.memset(ones_sb, 1.0)

        # v natural layout + bias via ones x bv matmul.
        def emit_v_chunk(t):
            ps = ps_pool.tile([128, 512], F32, tag="big", name="vps")
            for kc in range(8):
                nc.tensor.matmul(
                    ps,
                    xT_sb[:, kc, ts(t, 128)],
                    wv_sb[:, kc, :],
                    start=(kc == 0),
                    stop=False,
                )
            nc.tensor.matmul(ps, ones_sb, bv_sb, start=False, stop=True)
            vv = v_sb[:, t].rearrange("p (h e) -> p h e", e=E)
            nc.vector.tensor_copy(vv[:, :, 0:D], ps.rearrange("p (h d) -> p h d", d=D))
            nc.vector.memset(vv[:, :, D : D + 1], 1.0)

        for t in range(16):
            emit_v_chunk(t)

        # qk chunk m as a list of per-matmul thunks (so they can be spread
        # through an earlier pair's j-loop as PE filler).
        def qk_chunk_thunks(m):
            thunks = []
            for i4q in range(4):
                hold = {}

                def mk(kc, m=m, i4q=i4q, hold=hold):
                    def f():
                        if kc == 0:
                            hold["ps"] = ps_pool.tile([128, 512], F32, tag="big", name="qkps")
                        nc.tensor.matmul(
                            hold["ps"],
                            wqk_sb[:, kc, ts(m, 128)],
                            xT_sb[:, kc, ds(i4q * 512, 512)],
                            start=(kc == 0),
                            stop=(kc == 7),
                        )
                        if kc == 7:
                            nc.vector.tensor_scalar_add(
                                qkT_sb[:, m, ds(i4q * 512, 512)],
                                hold["ps"],
                                bqk_sb[:, ds(m, 1)],
                            )

                    return f

                for kc in range(8):
                    thunks.append(mk(kc))
            return thunks

        def emit_proj_quarter(i4):
            for t in range(4 * i4, 4 * i4 + 4):
                for nh in range(2):
                    pp = ps_pool.tile([128, 512], F32, tag="big", name="pjps")
                    for hc in range(4):
                        nc.tensor.matmul(
                            pp,
                            oT_sb[:, hc, ts(t, 128)],
                            wp_sb[:, hc, ds(nh * 512, 512)],
                            start=(hc == 0),
                            stop=(hc == 3),
                        )
                    ob = ob_pool.tile([128, 512], BF16)
                    nc.vector.tensor_copy(ob, pp)
                    nc.sync.dma_start(out_d[ts(t, 128), ds(nh * 512, 512)], ob)

        def emit_exp(ex, st, eng):
            if eng == "A":
                nc.scalar.activation(
                    ex, st, mybir.ActivationFunctionType.Exp, scale=0.125
                )
            else:
                nc.vector.tensor_scalar(
                    ex.bitcast(I16), st, SCH_A, SCH_C,
                    mybir.AluOpType.mult, mybir.AluOpType.add,
                )

        # Phase 2: per head PAIR. The two heads' S matmuls target different PE
        # row groups (base partitions 0 / 64) so back-to-back issue runs them
        # concurrently; both write one [128, 1024] st tile and share one exp.
        for p in range(4):
            qA, qB = qkT_sb[0:64, p], qkT_sb[64:128, p]
            kA, kB = qkT_sb[0:64, 4 + p], qkT_sb[64:128, 4 + p]
            hA, hB = 2 * p, 2 * p + 1
            # PE filler emitted between S and O each step: next pair's qk
            # matmuls (pairs 0..2), or earlier quarters' projections (pair 3).
            for th in qk_chunk_thunks(p):
                th()
            for th in qk_chunk_thunks(4 + p):
                th()
            for i4 in range(4):
                i0 = i4 * 512
                otA = ot_pool.tile([65, 512], F32, tag="ot")
                otB = ot_pool.tile([65, 512], F32, tag="ot")
                for j in range(16):
                    st = st_pool.tile([128, 1024], F32, tag="st")
                    nc.tensor.matmul(
                        st[:, 0:512], kA[:, ts(j, 128)], qA[:, ds(i0, 512)],
                        start=True, stop=True,
                    )
                    nc.tensor.matmul(
                        st[:, 512:1024], kB[:, ts(j, 128)], qB[:, ds(i0, 512)],
                        start=True, stop=True,
                    )
                    ex = exp_pool.tile([128, 1024], BF16)
                    emit_exp(ex, st, ENGINES[j])
                    if taps and p == 0 and i4 == 0 and j == 0:
                        nc.sync.dma_start(tap_ex, ex)
                    vvj = v_sb[:, j].rearrange("p (h e) -> p h e", e=E)
                    nc.tensor.matmul(
                        otA, vvj[:, hA], ex[:, 0:512], start=(j == 0), stop=(j == 15)
                    )
                    nc.tensor.matmul(
                        otB, vvj[:, hB], ex[:, 512:1024], start=(j == 0), stop=(j == 15)
                    )
                for hp_, ot in ((0, otA), (64, otB)):
                    # Copy the whole accumulator out first: frees the PSUM slot
                    # fast; same DVE cost as one row (partitions are parallel).
                    otc = misc.tile([65, 512], F32, tag="otc")
                    nc.vector.tensor_copy(otc, ot)
                    if taps and p == 0 and i4 == 0 and hp_ == 0:
                        nc.sync.dma_start(tap_ot, otc)
                    # Softmax denominators: lane-scatter so reciprocal runs on
                    # 128 lanes x 4 elems instead of 1 lane x 512 (DVE divide
                    # is ~8 cycles/elem serial per lane).
                    s_t = misc.tile([128, 4], F32, tag="sct")
                    nc.sync.dma_start(s_t, otc[64:65])
                    r_t = misc.tile([128, 4], F32, tag="rct")
                    nc.vector.reciprocal(r_t, s_t)
                    rec0 = misc.tile([1, 512], F32, tag="rec0")
                    nc.sync.dma_start(rec0, r_t)
                    recb = misc.tile([64, 512], F32, tag="recb")
                    nc.gpsimd.partition_broadcast(recb, rec0)
                    tmp = misc.tile([64, 512], BF16, tag="tmp")
                    nc.vector.tensor_mul(tmp, otc[0:64], recb)
                    nc.sync.dma_start(oT_sb[hp_ : hp_ + 64, p, ds(i0, 512)], tmp)
                if p == 3:
                    emit_proj_quarter(i4)

    nc.compile()
    return nc


_PROGRAM = None


def kernel(x, W_qkv, b_qkv, W_proj, b_proj):
    global _PROGRAM, LAST_RESULTS
    x = np.asarray(x, dtype=np.float32)
    W_qkv = np.asarray(W_qkv, dtype=np.float32)
    b_qkv = np.asarray(b_qkv, dtype=np.float32)
    W_proj = np.asarray(W_proj, dtype=np.float32)
    b_proj = np.asarray(b_proj, dtype=np.float32)

    if _PROGRAM is None:
        _PROGRAM = _build_program()
    nc = _PROGRAM

    in_maps = []
    for core in range(8):
        b, hg = core // 2, core % 2
        h0 = hg * HPC
        sl = slice(h0 * D, h0 * D + CD)
        wq = W_qkv[:, 0 * C :][:, sl]
        wk = W_qkv[:, 1 * C :][:, sl]
        wv = W_qkv[:, 2 * C :][:, sl]
        bq = b_qkv[0 * C :][sl]
        bk = b_qkv[1 * C :][sl]
        bv = b_qkv[2 * C :][sl]
        in_maps.append(
            {
                "xT": np.ascontiguousarray(x[b].T).astype(NP_BF16),
                "wqk": np.concatenate([wq, wk], axis=1).astype(NP_BF16),
                "wv": np.ascontiguousarray(wv).astype(NP_BF16),
                "bqk": np.concatenate([bq, bk]).reshape(8, 128).T.astype(np.float32).copy(),
                "bv": bv.reshape(1, CD).astype(NP_BF16),
                "wp": np.ascontiguousarray(W_proj[sl, :]).astype(NP_BF16),
            }
        )

    res = run_bass_kernel_spmd(nc, in_maps, list(range(8)))
    LAST_RESULTS = res
    out = np.empty((B, N, C), dtype=np.float32)
    for b in range(B):
        out[b] = (
            res.results[2 * b]["out"].astype(np.float32)
            + res.results[2 * b + 1]["out"].astype(np.float32)
            + b_proj[None, :]
        )
    return out


# revision 20
# speedup vs baseline: 1.0826x; 1.0826x over previous
"""Trainium2 Bass kernel: multi-head attention block (B=4, N=2048, C=1024, H=16).

Sharding: 8 cores = (batch b in 0..3) x (head-group hg in 0..1, 8 heads each).
Each core computes qkv for its heads, full attention for its heads over its
batch, and a partial projection (its 512 rows of W_proj). Host sums the two
partials per batch and adds b_proj.

Device layout (all matmuls bf16 inputs, fp32 PSUM accumulate):
  - q,k produced in transposed layout qkT[dim, token] so S^T = k-chunk.T @ q^T
    needs no on-chip transposes.
  - v produced in natural layout [token, 65*h] with a trailing ones column per
    head, so the O matmul lhsT=[v|ones] gives row 64 = softmax denominator and
    rows 0..63 = unnormalized o^T in one PSUM accumulation chain.
  - head PAIRS: the two heads' S matmuls sit at base partitions 0/64 (distinct
    PE row groups) and issue back-to-back, so they run concurrently. Each head
    half has its own [128,512] st PSUM tile (4-buf pool = 2-j run-ahead) and
    its own [128,512] ex tile, so each O matmul waits only on its own half.
  - exp halves alternate between ACT (exact, fused scale 1/8) and DVE
    (Schraudolph bit trick: bits = round(S*log2e*16 + 16250.5) as int16,
    bitcast bf16 = 2^(S*log2e/8), ~2% rms err). Neither engine saturates, so
    the j-loop stays PE-bound. (fp8/DoubleRow measured to net zero here: DR
    caps M at 64, costing the ones-row denominator what the 2x contraction
    saves.)
  - PSUM->SBUF copies (qk bias-add, o^T accumulators, proj outputs, v) run as
    ACT activations (Identity with per-partition bias / Copy), keeping the DVE
    queue short so DVE exps aren't head-of-line blocked.
  - proj quarters are emitted inside pair 3's i4 loop (PE filler); output
    stores are bf16 (host sums the two partials per batch in fp32).
"""

import os
import sys
from contextlib import ExitStack

import numpy as np
import ml_dtypes

import concourse.bass as bass
import concourse.tile as tile
from concourse import bacc, mybir
from concourse.bass import ds, ts
from concourse.bass_utils import run_bass_kernel_spmd

try:  # without the NTFF hook module, a stray BASS_TRACE=1 would crash the run
    from antenv.axon_hooks import get_axon_ntff_profile_hook  # noqa: F401
except ImportError:
    os.environ.setdefault("BASS_NEVER_TRACE", "1")

BF16 = mybir.dt.bfloat16
F32 = mybir.dt.float32
I16 = mybir.dt.int16
NP_BF16 = ml_dtypes.bfloat16

B, N, C = 4, 2048, 1024
H, D = 16, 64
HPC = 8            # heads per core
CD = HPC * D       # 512 local qkv dims per core
E = D + 1          # 65: 64 v dims + ones column

LOG2E = 1.4426950408889634
SCH_A = 128.0 * LOG2E / 8.0      # 23.083...
SCH_C = 16256.0 - 5.5            # bf16 exp2 bias with mid-point correction

LAST_RESULTS = None  # stash for test harness (exec_time_ns, trace paths)


def _build_program(taps=False):
    nc = bacc.Bacc("TRN2", target_bir_lowering=False, debug=False)

    xT_d = nc.dram_tensor("xT", [C, N], BF16, kind="ExternalInput").ap()
    wqk_d = nc.dram_tensor("wqk", [C, 2 * CD], BF16, kind="ExternalInput").ap()
    wv_d = nc.dram_tensor("wv", [C, CD], BF16, kind="ExternalInput").ap()
    bqk_d = nc.dram_tensor("bqk", [128, 8], F32, kind="ExternalInput").ap()
    bv_d = nc.dram_tensor("bv", [1, CD], BF16, kind="ExternalInput").ap()
    wp_d = nc.dram_tensor("wp", [CD, C], BF16, kind="ExternalInput").ap()
    out_d = nc.dram_tensor("out", [N, C], BF16, kind="ExternalOutput").ap()
    if taps:
        tap_ex = nc.dram_tensor("tap_ex", [128, 512], BF16, kind="ExternalOutput").ap()
        tap_ot = nc.dram_tensor("tap_ot", [65, 512], F32, kind="ExternalOutput").ap()

    with tile.TileContext(nc) as tc, ExitStack() as ctx:
        singles = ctx.enter_context(tc.tile_pool(name="singles", bufs=1))
        ps_pool = ctx.enter_context(tc.tile_pool(name="ps", bufs=2, space="PSUM"))
        st_pool = ctx.enter_context(tc.tile_pool(name="stp", bufs=4, space="PSUM"))
        ot_pool = ctx.enter_context(tc.tile_pool(name="ot", bufs=2, space="PSUM"))
        exp_pool = ctx.enter_context(tc.tile_pool(name="expp", bufs=8))
        misc = ctx.enter_context(tc.tile_pool(name="misc", bufs=4))
        ob_pool = ctx.enter_context(tc.tile_pool(name="ob", bufs=3))

        # Persistent SBUF tensors, chunk-major: [partition, chunk, free].
        xT_sb = singles.tile([128, 8, N], BF16)        # x^T   [c, token]
        wqk_sb = singles.tile([128, 8, 2 * CD], BF16)  # W_qk  [c, m]
        wv_sb = singles.tile([128, 8, CD], BF16)       # W_v   [c, n]
        bqk_sb = singles.tile([128, 8], F32)
        bv_sb = singles.tile([1, CD], BF16)
        ones_sb = singles.tile([1, 128], BF16)
        wp_sb = singles.tile([128, 4, C], BF16)        # W_proj [hd, n]
        qkT_sb = singles.tile([128, 8, N], BF16)       # chunks 0..3 = q, 4..7 = k
        v_sb = singles.tile([128, 16, HPC * E], BF16)  # [token-in-chunk, tchunk, h*(64+1)]
        oT_sb = singles.tile([128, 4, N], BF16)        # o^T, proj lhsT layout

        nc.sync.dma_start(wv_sb, wv_d.rearrange("(c p) m -> p c m", p=128))
        nc.sync.dma_start(bv_sb, bv_d)
        for kc in range(8):
            nc.sync.dma_start(xT_sb[:, kc], xT_d.rearrange("(c p) t -> p c t", p=128)[:, kc])
        nc.sync.dma_start(wqk_sb, wqk_d.rearrange("(c p) m -> p c m", p=128))
        nc.sync.dma_start(bqk_sb, bqk_d)
        nc.sync.dma_start(wp_sb, wp_d.rearrange("(c p) n -> p c n", p=128))
        nc.vector.memset(ones_sb, 1.0)

        # v natural layout + bias via ones x bv matmul.
        def emit_v_chunk(t):
            ps = ps_pool.tile([128, 512], F32, tag="big", name="vps")
            for kc in range(8):
                nc.tensor.matmul(
                    ps,
                    xT_sb[:, kc, ts(t, 128)],
                    wv_sb[:, kc, :],
                    start=(kc == 0),
                    stop=False,
                )
            nc.tensor.matmul(ps, ones_sb, bv_sb, start=False, stop=True)
            vv = v_sb[:, t].rearrange("p (h e) -> p h e", e=E)
            nc.scalar.activation(
                vv[:, :, 0:D], ps.rearrange("p (h d) -> p h d", d=D),
                mybir.ActivationFunctionType.Copy,
            )
            nc.vector.memset(vv[:, :, D : D + 1], 1.0)

        for t in range(16):
            emit_v_chunk(t)

        # qk chunk m as a list of per-matmul thunks (so they can be spread
        # through an earlier pair's j-loop as PE filler).
        def qk_chunk_thunks(m):
            thunks = []
            for i4q in range(4):
                hold = {}

                def mk(kc, m=m, i4q=i4q, hold=hold):
                    def f():
                        if kc == 0:
                            hold["ps"] = ps_pool.tile([128, 512], F32, tag="big", name="qkps")
                        nc.tensor.matmul(
                            hold["ps"],
                            wqk_sb[:, kc, ts(m, 128)],
                            xT_sb[:, kc, ds(i4q * 512, 512)],
                            start=(kc == 0),
                            stop=(kc == 7),
                        )
                        if kc == 7:
                            # bias-add + fp32->bf16 move on ACT (per-partition
                            # bias vector), keeping DVE free for exps.
                            nc.scalar.activation(
                                qkT_sb[:, m, ds(i4q * 512, 512)],
                                hold["ps"],
                                mybir.ActivationFunctionType.Identity,
                                bias=bqk_sb[:, ds(m, 1)],
                            )

                    return f

                for kc in range(8):
                    thunks.append(mk(kc))
            return thunks

        def emit_proj_quarter(i4):
            for t in range(4 * i4, 4 * i4 + 4):
                for nh in range(2):
                    pp = ps_pool.tile([128, 512], F32, tag="big", name="pjps")
                    for hc in range(4):
                        nc.tensor.matmul(
                            pp,
                            oT_sb[:, hc, ts(t, 128)],
                            wp_sb[:, hc, ds(nh * 512, 512)],
                            start=(hc == 0),
                            stop=(hc == 3),
                        )
                    ob = ob_pool.tile([128, 512], BF16)
                    nc.scalar.activation(ob, pp, mybir.ActivationFunctionType.Copy)
                    nc.sync.dma_start(out_d[ts(t, 128), ds(nh * 512, 512)], ob)

        def emit_exp(ex, st, eng):
            if eng == "A":
                nc.scalar.activation(
                    ex, st, mybir.ActivationFunctionType.Exp, scale=0.125
                )
            else:
                nc.vector.tensor_scalar(
                    ex.bitcast(I16), st, SCH_A, SCH_C,
                    mybir.AluOpType.mult, mybir.AluOpType.add,
                )

        # Phase 2: per head PAIR. The two heads' S matmuls target different PE
        # row groups (base partitions 0 / 64) so back-to-back issue runs them
        # concurrently into separate per-head st tiles.
        for p in range(4):
            qA, qB = qkT_sb[0:64, p], qkT_sb[64:128, p]
            kA, kB = qkT_sb[0:64, 4 + p], qkT_sb[64:128, 4 + p]
            hA, hB = 2 * p, 2 * p + 1
            # PE filler emitted between S and O each step: next pair's qk
            # matmuls (pairs 0..2), or earlier quarters' projections (pair 3).
            for th in qk_chunk_thunks(p):
                th()
            for th in qk_chunk_thunks(4 + p):
                th()
            for i4 in range(4):
                i0 = i4 * 512
                otA = ot_pool.tile([65, 512], F32, tag="ot")
                otB = ot_pool.tile([65, 512], F32, tag="ot")
                for j in range(16):
                    vvj = v_sb[:, j].rearrange("p (h e) -> p h e", e=E)
                    for half, (kh, qh, ot, h) in enumerate(
                        ((kA, qA, otA, hA), (kB, qB, otB, hB))
                    ):
                        st = st_pool.tile([128, 512], F32, tag="st")
                        nc.tensor.matmul(
                            st, kh[:, ts(j, 128)], qh[:, ds(i0, 512)],
                            start=True, stop=True,
                        )
                        ex = exp_pool.tile([128, 512], BF16, tag="ex")
                        emit_exp(ex, st, "A" if (j + half) % 2 == 0 else "D")
                        if taps and p == 0 and i4 == 0 and j == 0 and half == 0:
                            nc.sync.dma_start(tap_ex, ex)
                        nc.tensor.matmul(
                            ot, vvj[:, h], ex, start=(j == 0), stop=(j == 15)
                        )
                for hp_, ot in ((0, otA), (64, otB)):
                    # Move the accumulator out on ACT: frees the PSUM slot and
                    # keeps DVE short.
                    otc = misc.tile([65, 512], F32, tag="otc")
                    nc.scalar.activation(otc, ot, mybir.ActivationFunctionType.Copy)
                    if taps and p == 0 and i4 == 0 and hp_ == 0:
                        nc.sync.dma_start(tap_ot, otc)
                    # Softmax denominators: lane-scatter so reciprocal runs on
                    # 128 lanes x 4 elems instead of 1 lane x 512 (DVE divide
                    # is ~8 cycles/elem serial per lane).
                    s_t = misc.tile([128, 4], F32, tag="sct")
                    nc.sync.dma_start(s_t, otc[64:65])
                    r_t = misc.tile([128, 4], F32, tag="rct")
                    nc.vector.reciprocal(r_t, s_t)
                    rec0 = misc.tile([1, 512], F32, tag="rec0")
                    nc.sync.dma_start(rec0, r_t)
                    recb = misc.tile([64, 512], F32, tag="recb")
                    nc.gpsimd.partition_broadcast(recb, rec0)
                    tmp = misc.tile([64, 512], BF16, tag="tmp")
                    nc.vector.tensor_mul(tmp, otc[0:64], recb)
                    nc.sync.dma_start(oT_sb[hp_ : hp_ + 64, p, ds(i0, 512)], tmp)
                if p == 3:
                    emit_proj_quarter(i4)

    nc.compile()
    return nc


_PROGRAM = None


def kernel(x, W_qkv, b_qkv, W_proj, b_proj):
    global _PROGRAM, LAST_RESULTS
    x = np.asarray(x, dtype=np.float32)
    W_qkv = np.asarray(W_qkv, dtype=np.float32)
    b_qkv = np.asarray(b_qkv, dtype=np.float32)
    W_proj = np.asarray(W_proj, dtype=np.float32)
    b_proj = np.asarray(b_proj, dtype=np.float32)

    if _PROGRAM is None:
        _PROGRAM = _build_program()
    nc = _PROGRAM

    in_maps = []
    for core in range(8):
        b, hg = core // 2, core % 2
        h0 = hg * HPC
        sl = slice(h0 * D, h0 * D + CD)
        wq = W_qkv[:, 0 * C :][:, sl]
        wk = W_qkv[:, 1 * C :][:, sl]
        wv = W_qkv[:, 2 * C :][:, sl]
        bq = b_qkv[0 * C :][sl]
        bk = b_qkv[1 * C :][sl]
        bv = b_qkv[2 * C :][sl]
        in_maps.append(
            {
                "xT": np.ascontiguousarray(x[b].T).astype(NP_BF16),
                "wqk": np.concatenate([wq, wk], axis=1).astype(NP_BF16),
                "wv": np.ascontiguousarray(wv).astype(NP_BF16),
                "bqk": np.concatenate([bq, bk]).reshape(8, 128).T.astype(np.float32).copy(),
                "bv": bv.reshape(1, CD).astype(NP_BF16),
                "wp": np.ascontiguousarray(W_proj[sl, :]).astype(NP_BF16),
            }
        )

    res = run_bass_kernel_spmd(nc, in_maps, list(range(8)))
    LAST_RESULTS = res
    out = np.empty((B, N, C), dtype=np.float32)
    for b in range(B):
        out[b] = (
            res.results[2 * b]["out"].astype(np.float32)
            + res.results[2 * b + 1]["out"].astype(np.float32)
            + b_proj[None, :]
        )
    return out


# revision 22
# speedup vs baseline: 1.1602x; 1.0716x over previous
"""Trainium2 Bass kernel: multi-head attention block (B=4, N=2048, C=1024, H=16).

Sharding: 8 cores = (batch b in 0..3) x (head-group hg in 0..1, 8 heads each).
Each core computes qkv for its heads, full attention for its heads over its
batch, and a partial projection (its 512 rows of W_proj). Host sums the two
partials per batch and adds b_proj.

Device layout (all matmuls bf16 inputs, fp32 PSUM accumulate):
  - q,k produced in transposed layout qkT[dim, token] so S^T = k-chunk.T @ q^T
    needs no on-chip transposes.
  - v produced in natural layout [token, 65*h] with a trailing ones column per
    head, so the O matmul lhsT=[v|ones] gives row 64 = softmax denominator and
    rows 0..63 = unnormalized o^T in one PSUM accumulation chain.
  - exp on ScalarE with fused scale=1/8; no max subtraction (logits bounded).
  - head PAIRS: the two heads' S matmuls sit at base partitions 0/64 (distinct
    PE row groups) and issue back-to-back, so they run concurrently.

Scheduling: the attention j-loop is ACT(exp)-bound at ~1.11us/j, with the PE
needing only ~0.85us/j for S+O. ALL other PE work (v chunks, qk chunks, proj)
is queued as filler thunks and emitted just-in-time inside the j-loops, so the
ACT engine starts exping ~25us into the kernel and never starves. Without
this, v+qk run serially before the first exp (~74us of idle ACT) and proj
serially after the last (~30us) -- that's the baseline's 395us -> ~340us.
Only pair-0's first quarter runs PE-bound (~2us/j) while it produces the
remaining v chunks just ahead of their O matmuls.
"""

import os
import sys
from collections import deque
from contextlib import ExitStack

import numpy as np
import ml_dtypes

import concourse.bass as bass
import concourse.tile as tile
from concourse import bacc, mybir
from concourse.bass import ds, ts
from concourse.bass_utils import run_bass_kernel_spmd

try:  # without the NTFF hook module, a stray BASS_TRACE=1 would crash the run
    from antenv.axon_hooks import get_axon_ntff_profile_hook  # noqa: F401
except ImportError:
    os.environ.setdefault("BASS_NEVER_TRACE", "1")

BF16 = mybir.dt.bfloat16
F32 = mybir.dt.float32
NP_BF16 = ml_dtypes.bfloat16

B, N, C = 4, 2048, 1024
H, D = 16, 64
HPC = 8            # heads per core
CD = HPC * D       # 512 local qkv dims per core
E = D + 1          # 65: 64 v dims + ones column

LAST_RESULTS = None  # stash for test harness (exec_time_ns, trace paths)


def _build_program(taps=False):
    nc = bacc.Bacc("TRN2", target_bir_lowering=False, debug=False)

    xT_d = nc.dram_tensor("xT", [C, N], BF16, kind="ExternalInput").ap()
    wqk_d = nc.dram_tensor("wqk", [C, 2 * CD], BF16, kind="ExternalInput").ap()
    wv_d = nc.dram_tensor("wv", [C, CD], BF16, kind="ExternalInput").ap()
    bqk_d = nc.dram_tensor("bqk", [128, 8], F32, kind="ExternalInput").ap()
    bv_d = nc.dram_tensor("bv", [1, CD], BF16, kind="ExternalInput").ap()
    wp_d = nc.dram_tensor("wp", [CD, C], BF16, kind="ExternalInput").ap()
    out_d = nc.dram_tensor("out", [N, C], BF16, kind="ExternalOutput").ap()
    if taps:
        tap_ex = nc.dram_tensor("tap_ex", [128, 1024], BF16, kind="ExternalOutput").ap()
        tap_ot = nc.dram_tensor("tap_ot", [65, 512], F32, kind="ExternalOutput").ap()

    with tile.TileContext(nc) as tc, ExitStack() as ctx:
        singles = ctx.enter_context(tc.tile_pool(name="singles", bufs=1))
        ps_pool = ctx.enter_context(tc.tile_pool(name="ps", bufs=2, space="PSUM"))
        st_pool = ctx.enter_context(tc.tile_pool(name="stp", bufs=2, space="PSUM"))
        ot_pool = ctx.enter_context(tc.tile_pool(name="ot", bufs=2, space="PSUM"))
        exp_pool = ctx.enter_context(tc.tile_pool(name="expp", bufs=4))
        misc = ctx.enter_context(tc.tile_pool(name="misc", bufs=4))
        ob_pool = ctx.enter_context(tc.tile_pool(name="ob", bufs=3))

        # Persistent SBUF tensors, chunk-major: [partition, chunk, free].
        xT_sb = singles.tile([128, 8, N], BF16)        # x^T   [c, token]
        wqk_sb = singles.tile([128, 8, 2 * CD], BF16)  # W_qk  [c, m]
        wv_sb = singles.tile([128, 8, CD], BF16)       # W_v   [c, n]
        bqk_sb = singles.tile([128, 8], F32)
        bv_sb = singles.tile([1, CD], BF16)
        ones_sb = singles.tile([1, 128], BF16)
        wp_sb = singles.tile([128, 4, C], BF16)        # W_proj [hd, n]
        qkT_sb = singles.tile([128, 8, N], BF16)       # chunks 0..3 = q, 4..7 = k
        v_sb = singles.tile([128, 16, HPC * E], BF16)  # [token-in-chunk, tchunk, h*(64+1)]
        oT_sb = singles.tile([128, 4, N], BF16)        # o^T, proj lhsT layout

        nc.sync.dma_start(wv_sb, wv_d.rearrange("(c p) m -> p c m", p=128))
        nc.sync.dma_start(bv_sb, bv_d)
        for kc in range(8):
            nc.sync.dma_start(xT_sb[:, kc], xT_d.rearrange("(c p) t -> p c t", p=128)[:, kc])
        nc.sync.dma_start(wqk_sb, wqk_d.rearrange("(c p) m -> p c m", p=128))
        nc.sync.dma_start(bqk_sb, bqk_d)
        nc.sync.dma_start(wp_sb, wp_d.rearrange("(c p) n -> p c n", p=128))
        nc.vector.memset(ones_sb, 1.0)

        # ---- filler thunks -------------------------------------------------
        # v natural layout + bias via ones x bv matmul.
        def emit_v_chunk(t):
            ps = ps_pool.tile([128, 512], F32, tag="big", name="vps")
            for kc in range(8):
                nc.tensor.matmul(
                    ps,
                    xT_sb[:, kc, ts(t, 128)],
                    wv_sb[:, kc, :],
                    start=(kc == 0),
                    stop=False,
                )
            nc.tensor.matmul(ps, ones_sb, bv_sb, start=False, stop=True)
            vv = v_sb[:, t].rearrange("p (h e) -> p h e", e=E)
            nc.vector.tensor_copy(vv[:, :, 0:D], ps.rearrange("p (h d) -> p h d", d=D))
            nc.vector.memset(vv[:, :, D : D + 1], 1.0)

        # one qk sub-chunk: 128 qkv-dims (chunk m) x 512 tokens (quarter i4q)
        def emit_qk_sub(m, i4q):
            ps = ps_pool.tile([128, 512], F32, tag="big", name="qkps")
            for kc in range(8):
                nc.tensor.matmul(
                    ps,
                    wqk_sb[:, kc, ts(m, 128)],
                    xT_sb[:, kc, ds(i4q * 512, 512)],
                    start=(kc == 0),
                    stop=(kc == 7),
                )
            nc.vector.tensor_scalar_add(
                qkT_sb[:, m, ds(i4q * 512, 512)],
                ps,
                bqk_sb[:, ds(m, 1)],
            )

        # one proj chain: 128 tokens (chunk t) x 512 out-cols (half nh)
        def emit_proj_chain(t, nh):
            pp = ps_pool.tile([128, 512], F32, tag="big", name="pjps")
            for hc in range(4):
                nc.tensor.matmul(
                    pp,
                    oT_sb[:, hc, ts(t, 128)],
                    wp_sb[:, hc, ds(nh * 512, 512)],
                    start=(hc == 0),
                    stop=(hc == 3),
                )
            ob = ob_pool.tile([128, 512], BF16)
            nc.vector.tensor_copy(ob, pp)
            nc.sync.dma_start(out_d[ts(t, 128), ds(nh * 512, 512)], ob)

        fillers = deque()

        def drain_fillers(n):
            while n > 0 and fillers:
                fillers.popleft()()
                n -= 1

        # ---- prefix: the minimum needed for pair 0 / i4 0 / j 0 ------------
        for t in range(4):
            emit_v_chunk(t)
        emit_qk_sub(0, 0)   # q of pair 0, queries 0:512
        emit_qk_sub(4, 0)   # k of pair 0, keys 0:512 (j 0..3)

        # queue: v 4..15 and remaining pair-0 k subs early (just-in-time for
        # the ramp quarter), then pair-0 q subs, then later pairs' qk subs.
        ramp = deque()
        for item in ("v4", "k1", "v5", "v6", "k2", "v7", "v8", "k3",
                     "v9", "v10", "q1", "v11", "v12", "v13", "v14", "v15"):
            if item[0] == "v":
                ramp.append(lambda t=int(item[1:]): emit_v_chunk(t))
            elif item[0] == "k":
                ramp.append(lambda s=int(item[1:]): emit_qk_sub(4, s))
            else:
                ramp.append(lambda s=int(item[1:]): emit_qk_sub(0, s))
        for s in (2, 3):
            fillers.append(lambda s=s: emit_qk_sub(0, s))
        for p_next in (1, 2, 3):
            for s in range(4):
                fillers.append(lambda p=p_next, s=s: emit_qk_sub(p, s))
                fillers.append(lambda p=p_next, s=s: emit_qk_sub(4 + p, s))

        # ---- attention: per head pair, per query-quarter, per key-chunk ----
        for p in range(4):
            qA, qB = qkT_sb[0:64, p], qkT_sb[64:128, p]
            kA, kB = qkT_sb[0:64, 4 + p], qkT_sb[64:128, 4 + p]
            hA, hB = 2 * p, 2 * p + 1
            for i4 in range(4):
                i0 = i4 * 512
                otA = ot_pool.tile([65, 512], F32, tag="ot")
                otB = ot_pool.tile([65, 512], F32, tag="ot")
                budget = 0.0
                for j in range(16):
                    st = st_pool.tile([128, 1024], F32, tag="st")
                    nc.tensor.matmul(
                        st[:, 0:512], kA[:, ts(j, 128)], qA[:, ds(i0, 512)],
                        start=True, stop=True,
                    )
                    nc.tensor.matmul(
                        st[:, 512:1024], kB[:, ts(j, 128)], qB[:, ds(i0, 512)],
                        start=True, stop=True,
                    )
                    ex = exp_pool.tile([128, 1024], BF16)
                    nc.scalar.activation(
                        ex, st, mybir.ActivationFunctionType.Exp, scale=0.125
                    )
                    if taps and p == 0 and i4 == 0 and j == 0:
                        nc.sync.dma_start(tap_ex, ex)
                    vvj = v_sb[:, j].rearrange("p (h e) -> p h e", e=E)
                    nc.tensor.matmul(
                        otA, vvj[:, hA], ex[:, 0:512], start=(j == 0), stop=(j == 15)
                    )
                    nc.tensor.matmul(
                        otB, vvj[:, hB], ex[:, 512:1024], start=(j == 0), stop=(j == 15)
                    )
                    if p == 0 and i4 == 0:
                        if ramp:
                            ramp.popleft()()  # ~2 thunks/j keeps v just ahead
                        if ramp:
                            ramp.popleft()()
                    else:
                        budget += 0.5 if p == 3 else 0.22
                        while budget >= 1.0 and fillers:
                            fillers.popleft()()
                            budget -= 1.0
                for hp_, ot in ((0, otA), (64, otB)):
                    # Copy the whole accumulator out first: frees the PSUM slot
                    # fast; same DVE cost as one row (partitions are parallel).
                    otc = misc.tile([65, 512], F32, tag="otc")
                    nc.vector.tensor_copy(otc, ot)
                    if taps and p == 0 and i4 == 0 and hp_ == 0:
                        nc.sync.dma_start(tap_ot, otc)
                    # Softmax denominators: lane-scatter so reciprocal runs on
                    # 128 lanes x 4 elems instead of 1 lane x 512 (DVE divide
                    # is ~8 cycles/elem serial per lane).
                    s_t = misc.tile([128, 4], F32, tag="sct")
                    nc.sync.dma_start(s_t, otc[64:65])
                    r_t = misc.tile([128, 4], F32, tag="rct")
                    nc.vector.reciprocal(r_t, s_t)
                    rec0 = misc.tile([1, 512], F32, tag="rec0")
                    nc.sync.dma_start(rec0, r_t)
                    recb = misc.tile([64, 512], F32, tag="recb")
                    nc.gpsimd.partition_broadcast(recb, rec0)
                    tmp = misc.tile([64, 512], BF16, tag="tmp")
                    nc.vector.tensor_mul(tmp, otc[0:64], recb)
                    nc.sync.dma_start(oT_sb[hp_ : hp_ + 64, p, ds(i0, 512)], tmp)
                # after pair-3 quarter q completes, its proj columns unblock
                if p == 3 and i4 >= 1:
                    for t in range(4 * (i4 - 1), 4 * i4):
                        for nh in range(2):
                            fillers.append(lambda t=t, nh=nh: emit_proj_chain(t, nh))

        drain_fillers(len(fillers))
        for t in range(12, 16):
            for nh in range(2):
                emit_proj_chain(t, nh)

    nc.compile()
    return nc


_PROGRAM = None


def kernel(x, W_qkv, b_qkv, W_proj, b_proj):
    global _PROGRAM, LAST_RESULTS
    x = np.asarray(x, dtype=np.float32)
    W_qkv = np.asarray(W_qkv, dtype=np.float32)
    b_qkv = np.asarray(b_qkv, dtype=np.float32)
    W_proj = np.asarray(W_proj, dtype=np.float32)
    b_proj = np.asarray(b_proj, dtype=np.float32)

    if _PROGRAM is None:
        _PROGRAM = _build_program()
    nc = _PROGRAM

    in_maps = []
    for core in range(8):
        b, hg = core // 2, core % 2
        h0 = hg * HPC
        sl = slice(h0 * D, h0 * D + CD)
        wq = W_qkv[:, 0 * C :][:, sl]
        wk = W_qkv[:, 1 * C :][:, sl]
        wv = W_qkv[:, 2 * C :][:, sl]
        bq = b_qkv[0 * C :][sl]
        bk = b_qkv[1 * C :][sl]
        bv = b_qkv[2 * C :][sl]
        in_maps.append(
            {
                "xT": np.ascontiguousarray(x[b].T).astype(NP_BF16),
                "wqk": np.concatenate([wq, wk], axis=1).astype(NP_BF16),
                "wv": np.ascontiguousarray(wv).astype(NP_BF16),
                "bqk": np.concatenate([bq, bk]).reshape(8, 128).T.astype(np.float32).copy(),
                "bv": bv.reshape(1, CD).astype(NP_BF16),
                "wp": np.ascontiguousarray(W_proj[sl, :]).astype(NP_BF16),
            }
        )

    res = run_bass_kernel_spmd(nc, in_maps, list(range(8)))
    LAST_RESULTS = res
    out = np.empty((B, N, C), dtype=np.float32)
    for b in range(B):
        out[b] = (
            res.results[2 * b]["out"].astype(np.float32)
            + res.results[2 * b + 1]["out"].astype(np.float32)
            + b_proj[None, :]
        )
    return out
